# revision 30
# baseline (speedup 1.0000x reference)
"""Trainium2 Bass kernel for NeuroVPR Vanilla SNN (3-layer LIF, T=3).

Data-parallel over batch: B=16384 -> 2048 per core x 8 cores.

Math (per timestep, per layer): v = (v_prev + h)/2; s = (v>=1); v *= (1-s).
Weights are scaled by 16 on host (keeps fp8-e4m3 values out of the subnormal
range), so PSUM holds 16h and we track W = 32*v:
    W_t = M_{t-1}/2 + psum_t    (psum = 16*h)
    s_t = (W_t >= 32)
    M_t = W_t * (W_t < 32) = (s_t < 0.5) * W_t
All scalings are powers of two, so spike decisions match the fp32 recurrence
up to matmul quantization error. Layer-2 membranes peak at 0.59 of threshold
under fp8 quantization (validated on host), so the output spikes are
insensitive to the fp8 rounding.

Matmuls run in fp8-e4m3 with perf_mode=DoubleRow: contraction tiles of 256
rows packed as [128 partitions, 2, free]; 2 MACs/PE/cycle. D is padded
2752->2816 = 11*256; pad row 2752 carries the L1 bias (x=1 there). The x
input is fp8, halving HBM traffic vs fp16.

LIF runs PSUM-resident everywhere: for t>=1, ScalarE preloads each bank
with M/2 and the matmul group accumulates on top with start=False (the
has_written bits survive from the previous full-bank group; L3 stationary
operands are padded to 128 rows so every group writes its full bank). The
bank then holds W directly: VectorE does s = (psum >= 32) and
m = (s < 0.5)*psum (one PSUM operand per instruction - the DVE cannot read
PSUM twice). At t=0 the membrane is zero, so no preload and start=True.

Schedule: the PE runs instructions in order, so layer-2/3 groups must never
bunch up behind PSUM-bank recycling (that head-blocks the queue, idles the
PE >3.4us, and HAM halves the PE clock). Each timestep's L2/L3 work is
split into 2-bank "pairs" interleaved as slots inside later L1 contraction
loops, as early as their spike inputs allow. GpSimd has no ALU or PSUM path
on this target; its DMA queue is kept empty (a queued DMA there causes a
12us DRAIN stall mid-kernel).
"""
import os
import numpy as np

B, T, D = 16384, 3, 2752
DP = 2816           # D padded to 11*256 (pad row 2752 = bias row)
H, O = 256, 100
NCORES = 8
BC = B // NCORES    # 2048
NB = 512            # psum block along batch
KT2 = DP // 256     # 11 double-row contraction tiles for L1
HB = BC // 2        # half-batch per L1 pass (1024)

_compiled = {}
last_results = None  # BassKernelResults of the most recent run (for profiling)


def _build(use_b2, use_b3):
    from contextlib import ExitStack
    import concourse.bass as bass
    import concourse.mybir as mybir
    import concourse.tile as tile
    from concourse import bacc

    f8, f16, f32 = mybir.dt.float8e4, mybir.dt.float16, mybir.dt.float32
    A = mybir.AluOpType
    DR = mybir.MatmulPerfMode.DoubleRow

    nc = bacc.Bacc("TRN2", target_bir_lowering=False, debug=False)
    x = nc.dram_tensor("x", [T, KT2, 128, 2, BC], f8, kind="ExternalInput").ap()
    # w1 free layout (i, kk, m): [p, i*(KT2*H) + kk*H + m] = 16*W1T[kk*256+i*128+p, m]
    w1 = nc.dram_tensor("w1", [128, 2 * KT2 * H], f8, kind="ExternalInput").ap()
    w2 = nc.dram_tensor("w2", [128, 2 * H], f8, kind="ExternalInput").ap()
    w3 = nc.dram_tensor("w3", [128, 2 * 128], f8, kind="ExternalInput").ap()
    if use_b2:
        b2 = nc.dram_tensor("b2", [1, H], f16, kind="ExternalInput").ap()
    if use_b3:
        b3 = nc.dram_tensor("b3", [1, O], f16, kind="ExternalInput").ap()
    out = nc.dram_tensor("out", [O, BC], f32, kind="ExternalOutput").ap()

    W1C = KT2 * H  # column span of one i-group in w1t

    with tile.TileContext(nc) as tc, ExitStack() as ctx:
        wp = ctx.enter_context(tc.tile_pool(name="wp", bufs=1))
        xp = ctx.enter_context(tc.tile_pool(name="xp", bufs=24))
        pp1 = ctx.enter_context(tc.tile_pool(name="pp1", bufs=5, space="PSUM"))
        pp23 = ctx.enter_context(tc.tile_pool(name="pp23", bufs=3, space="PSUM"))
        sp = ctx.enter_context(tc.tile_pool(name="sp", bufs=1))
        tp = ctx.enter_context(tc.tile_pool(name="tp", bufs=6))

        # Warm-up: a few dummy DoubleRow matmuls on zeroed tiles keep the PE
        # busy from the preamble until the first real matmul's x tile lands,
        # so the HAM clock gate opens early and every real matmul runs at
        # 2.4 GHz (cold-start otherwise costs ~5us at half clock). They run
        # at the cold clock (~630ns each); too many head-block the queue.
        zw = wp.tile([128, 256], f8)
        zx = wp.tile([128, 2 * NB], f8)
        nc.vector.memset(zw[:, :], 0.0)
        nc.vector.memset(zx[:, :], 0.0)
        zw3 = zw[:, :].rearrange("p (i m) -> p i m", i=2)
        zx3 = zx[:, :].rearrange("p (i n) -> p i n", i=2)
        zp = pp23.tile([128, NB], f32, tag="ps23", name="warmup")
        for _ in range(10):
            nc.tensor.matmul(zp[:, :], zw3, zx3, start=True, stop=True,
                             perf_mode=DR)

        # resident weights
        w1t = wp.tile([128, 2 * W1C], f8)
        # kk=0 chunks first on the sync queue so matmuls can start ASAP
        nc.sync.dma_start(out=w1t[:, 0:H], in_=w1[:, 0:H])
        nc.sync.dma_start(out=w1t[:, W1C:W1C + H], in_=w1[:, W1C:W1C + H])
        nc.scalar.dma_start(out=w1t[:, H:W1C], in_=w1[:, H:W1C])
        nc.scalar.dma_start(out=w1t[:, W1C + H:2 * W1C], in_=w1[:, W1C + H:2 * W1C])
        w2t = wp.tile([128, 2 * H], f8)
        nc.scalar.dma_start(out=w2t[:, :], in_=w2[:, :])
        w3t = wp.tile([128, 2 * 128], f8)
        nc.scalar.dma_start(out=w3t[:, :], in_=w3[:, :])
        if use_b2 or use_b3:
            ones = wp.tile([1, NB], f16)
            nc.vector.memset(ones[:, :], 1.0)
        if use_b2:
            b2t = wp.tile([1, H], f16)
            nc.scalar.dma_start(out=b2t[:, :], in_=b2[:, :])
        if use_b3:
            b3t = wp.tile([1, O], f16)
            nc.scalar.dma_start(out=b3t[:, :], in_=b3[:, :])

        w1v = w1t[:, :].rearrange("p (i c) -> p i c", i=2)
        w2v = w2t[:, :].rearrange("p (i m) -> p i m", i=2)
        w3v = w3t[:, :].rearrange("p (i m) -> p i m", i=2)

        # persistent state (M = 16*m, no init needed: t=0 skips the M read)
        m1 = [sp.tile([128, BC], f16, tag=f"m1_{h}", name=f"m1_{h}") for h in range(2)]
        m2 = [sp.tile([128, BC], f16, tag=f"m2_{h}", name=f"m2_{h}") for h in range(2)]
        m3 = sp.tile([128, BC], f16, tag="m3")
        # spikes, double-buffered by timestep parity; layout [p, i*BC + n]
        s1 = [sp.tile([128, 2 * BC], f8, tag=f"s1_{j}", name=f"s1_{j}") for j in range(2)]
        s2 = [sp.tile([128, 2 * BC], f8, tag=f"s2_{j}", name=f"s2_{j}") for j in range(2)]
        outsb = sp.tile([128, BC], f32, tag="outsb")

        s1v = [s1[j][:, :].rearrange("p (i n) -> p i n", i=2) for j in range(2)]
        s2v = [s2[j][:, :].rearrange("p (i n) -> p i n", i=2) for j in range(2)]

        def l2_pair(t, h, bp, pool, tag, bounce=False):
            """Two adjacent batch blocks of layer 2 via PSUM-preload.

            bounce=True (in-pass pends at T-1): ScalarE copies the banks into
            a fp16 tile so VectorE does one merged 1024-wide compare. Tail
            pairs keep the direct per-bank compare (shortest chain)."""
            def emit():
                ps = []
                for b in (bp, bp + 1):
                    p = pool.tile([128, NB], f32, tag=tag, name=f"ps2_{t}_{h}_{b}")
                    ps.append(p)
                    if t > 0:
                        nc.scalar.mul(p[:, :], m2[h][:, b * NB:(b + 1) * NB], 0.5)
                for j, b in enumerate((bp, bp + 1)):
                    first = t == 0
                    if use_b2:
                        nc.tensor.matmul(ps[j][:, :], b2t[0:1, h * 128:(h + 1) * 128],
                                         ones[0:1, :], start=first, stop=False,
                                         skip_group_check=not first)
                        first = False
                    nc.tensor.matmul(
                        ps[j][:, :], w2v[:, :, h * 128:(h + 1) * 128],
                        s1v[t % 2][:, :, b * NB:(b + 1) * NB],
                        start=first, stop=True, skip_group_check=t > 0,
                        perf_mode=DR)
                if bounce and t == T - 1:
                    wpr = tp.tile([128, 2 * NB], f16, tag="w", name="w")
                    for j in range(2):
                        nc.scalar.copy(wpr[:, j * NB:(j + 1) * NB], ps[j][:, :])
                    c0 = h * BC + bp * NB
                    nc.vector.tensor_scalar(s2[t % 2][:, c0:c0 + 2 * NB],
                                            wpr[:, :], 32.0, None, A.is_ge)
                    return
                for j, b in enumerate((bp, bp + 1)):
                    c0 = h * BC + b * NB
                    nc.vector.tensor_scalar(s2[t % 2][:, c0:c0 + NB], ps[j][:, :],
                                            32.0, None, A.is_ge)
                if t != T - 1:
                    for j, b in enumerate((bp, bp + 1)):
                        c0 = h * BC + b * NB
                        nc.vector.scalar_tensor_tensor(
                            m2[h][:, b * NB:(b + 1) * NB],
                            s2[t % 2][:, c0:c0 + NB], 0.5, ps[j][:, :],
                            A.is_lt, A.mult)
            return emit

        def l3_pair(t, bp, pool, tag):
            """Layer-3 pair; stationary padded to 128 rows so the matmul
            writes the full bank (keeps has_written set for later preloads).
            At t<T-1 only the membrane update is needed: ScalarE bounces the
            banks to a fp16 tile, VectorE does one merged 1024-wide update."""
            def emit():
                ps = []
                for b in (bp, bp + 1):
                    p = pool.tile([128, NB], f32, tag=tag, name=f"ps3_{t}_{b}")
                    ps.append(p)
                    if t > 0:
                        nc.scalar.mul(p[:O, :], m3[:O, b * NB:(b + 1) * NB], 0.5)
                for j, b in enumerate((bp, bp + 1)):
                    first = t == 0
                    if use_b3:
                        nc.tensor.matmul(ps[j][:O, :], b3t[0:1, :], ones[0:1, :],
                                         start=first, stop=False,
                                         skip_group_check=not first)
                        first = False
                    nc.tensor.matmul(
                        ps[j][:, :], w3v[:, :, :],
                        s2v[t % 2][:, :, b * NB:(b + 1) * NB],
                        start=first, stop=True, skip_group_check=t > 0,
                        perf_mode=DR)
                if t == T - 1:
                    for j, b in enumerate((bp, bp + 1)):
                        bs = slice(b * NB, (b + 1) * NB)
                        nc.vector.tensor_scalar(outsb[:O, bs], ps[j][:O, :],
                                                32.0, None, A.is_ge)
                        nc.sync.dma_start(out=out[:, bs], in_=outsb[:O, bs])
                else:
                    wpr = tp.tile([128, 2 * NB], f16, tag="w", name="w")
                    for j in range(2):
                        nc.scalar.copy(wpr[:O, j * NB:(j + 1) * NB], ps[j][:O, :])
                    wv = wpr[:O, :]
                    nc.vector.scalar_tensor_tensor(
                        m3[:O, bp * NB:(bp + 2) * NB], wv, 32.0, wv,
                        A.is_lt, A.mult)
            return emit

        def l1_pass(t, half, pends=(), slots=()):
            """One half-batch L1 pass: 4 psum groups (2h x 2b), kk inner.
            Pending L2/L3 pair closures are emitted at the given kk slots.
            For t>=1 the banks are preloaded with M/2 and the whole
            accumulation group runs with start=False."""
            boff = half * HB
            pends = list(pends)
            slots = sorted(slots)[:len(pends)]
            ps1 = [[pp1.tile([128, NB], f32, tag="ps1", name=f"ps1_{t}_{half}_{h}_{b}")
                    for b in range(2)] for h in range(2)]
            last_t = t == T - 1
            if last_t:
                # no membrane write at T-1: preload M/2 so the bank holds W
                # after the matmuls and a single per-bank compare finishes
                # the pass (shortest possible tail chain)
                for h in range(2):
                    for b in range(2):
                        bs = slice(boff + b * NB, boff + (b + 1) * NB)
                        nc.scalar.mul(ps1[h][b][:, :], m1[h][:, bs], 0.5)
            for k in range(KT2):
                xt = xp.tile([128, 2 * HB], f8, tag="x", name="xt")
                xt3 = xt[:, :].rearrange("p (i n) -> p i n", i=2)
                nc.sync.dma_start(out=xt3, in_=x[t, k, :, :, boff:boff + HB])
                for h in range(2):
                    lhsT = w1v[:, :, k * H + h * 128: k * H + h * 128 + 128]
                    for b in range(2):
                        nc.tensor.matmul(
                            ps1[h][b][:, :], lhsT, xt3[:, :, b * NB:(b + 1) * NB],
                            start=(k == 0 and not last_t), stop=(k == KT2 - 1),
                            skip_group_check=last_t, perf_mode=DR)
                if slots and k == slots[0]:
                    slots.pop(0)
                    pends.pop(0)()
            if last_t:
                # per-bank spikes, b-major so the tail's l2 inputs finish first
                for b in range(2):
                    for h in range(2):
                        c0 = h * BC + boff + b * NB
                        nc.vector.tensor_scalar(s1[t % 2][:, c0:c0 + NB],
                                                ps1[h][b][:, :], 32.0, None,
                                                A.is_ge)
            else:
                # W into fp16 tiles first (releases all 4 banks quickly),
                # then merged 1024-wide spike/membrane ops
                wh = [tp.tile([128, 2 * NB], f16, tag="w", name="w")
                      for _ in range(2)]
                for h in range(2):
                    for b in range(2):
                        bs = slice(boff + b * NB, boff + (b + 1) * NB)
                        dst = wh[h][:, b * NB:(b + 1) * NB]
                        if t == 0:
                            nc.scalar.copy(dst, ps1[h][b][:, :])
                        else:
                            nc.vector.scalar_tensor_tensor(
                                dst, m1[h][:, bs], 0.5, ps1[h][b][:, :],
                                A.mult, A.add)
                for h in range(2):
                    c0 = h * BC + boff
                    nc.vector.tensor_scalar(s1[t % 2][:, c0:c0 + HB],
                                            wh[h][:, :], 32.0, None, A.is_ge)
                if t != T - 1:
                    for h in range(2):
                        wv = wh[h][:, :]
                        nc.vector.scalar_tensor_tensor(
                            m1[h][:, boff:boff + HB], wv, 32.0, wv,
                            A.is_lt, A.mult)
            for p in pends:  # leftovers (shouldn't happen with matched slots)
                p()

        # Pair placement: as early as spike inputs allow, so the final pass
        # carries little and the tail stays short.
        l1_pass(0, 0)
        l1_pass(0, 1)
        l1_pass(1, 0,
                [l2_pair(0, 0, 0, pp23, "ps23"),
                 l2_pair(0, 1, 0, pp23, "ps23"),
                 l2_pair(0, 0, 2, pp23, "ps23")], (2, 5, 8))
        l1_pass(1, 1,
                [l2_pair(0, 1, 2, pp23, "ps23"),
                 l3_pair(0, 0, pp23, "ps23"),
                 l3_pair(0, 2, pp23, "ps23")], (2, 5, 8))
        l1_pass(2, 0,
                [l2_pair(1, 0, 0, pp23, "ps23"),
                 l2_pair(1, 1, 0, pp23, "ps23"),
                 l2_pair(1, 0, 2, pp23, "ps23"),
                 l2_pair(1, 1, 2, pp23, "ps23")], (1, 3, 5, 8))
        l1_pass(2, 1,
                [l3_pair(1, 0, pp23, "ps23"),
                 l3_pair(1, 2, pp23, "ps23"),
                 l2_pair(2, 0, 0, pp23, "ps23", bounce=True),
                 l2_pair(2, 1, 0, pp23, "ps23", bounce=True)], (1, 4, 6, 8))
        # tail
        l3_pair(2, 0, pp23, "ps23")()
        l2_pair(2, 0, 2, pp1, "ps1")()
        l2_pair(2, 1, 2, pp1, "ps1")()
        l3_pair(2, 2, pp23, "ps23")()

    nc.compile()
    return nc


def kernel(dvs, W1, b1, W2, b2, W3, b3):
    global last_results
    import ml_dtypes
    from concourse.bass_utils import run_bass_kernel_spmd

    f8 = ml_dtypes.float8_e4m3
    use_b2 = bool(np.any(b2))
    use_b3 = bool(np.any(b3))
    key = (use_b2, use_b3)
    if key not in _compiled:
        _compiled[key] = _build(use_b2, use_b3)
    nc = _compiled[key]

    one8 = np.float32(1.0).astype(f8).view(np.uint8).item()
    # x: [B, T, D] -> fp8 [T, KT2, 128, 2, B]; pad row D=2752 carries bias (x=1)
    Xq = np.empty((T, DP, B), np.uint8)
    for t in range(T):
        Xq[t, :D, :] = np.asarray(dvs[:, t, :]).astype(f8).view(np.uint8).T
        Xq[t, D, :] = one8
        Xq[t, D + 1:, :] = 0
    Xr = Xq.reshape(T, KT2, 2, 128, B).swapaxes(2, 3)  # [T, KT2, 128, 2, B] view

    # weights scaled by 16, packed [p, i, ...] for DoubleRow
    W1p = np.zeros((DP, H), np.float32)
    W1p[:D] = W1.T * 16.0
    W1p[D] = b1 * 16.0
    w1q = W1p.astype(f8).reshape(KT2, 2, 128, H)
    w1dr = np.ascontiguousarray(
        w1q.transpose(2, 1, 0, 3).reshape(128, 2 * KT2 * H))
    w2dr = np.ascontiguousarray(
        (W2.T * 16.0).astype(f8).reshape(2, 128, H).transpose(1, 0, 2).reshape(128, 2 * H))
    W3p = np.zeros((H, 128), np.float32)
    W3p[:, :O] = W3.T * 16.0
    w3dr = np.ascontiguousarray(
        W3p.astype(f8).reshape(2, 128, 128).transpose(1, 0, 2).reshape(128, 256))

    in_maps = []
    for c in range(NCORES):
        xc = np.ascontiguousarray(Xr[:, :, :, :, c * BC:(c + 1) * BC]).view(f8)
        m = {"x": xc, "w1": w1dr, "w2": w2dr, "w3": w3dr}
        if use_b2:
            m["b2"] = (b2 * 16.0).astype(np.float16).reshape(1, H)
        if use_b3:
            m["b3"] = (b3 * 16.0).astype(np.float16).reshape(1, O)
        in_maps.append(m)

    trace = bool(os.environ.get("SNN_TRACE"))
    last_results = run_bass_kernel_spmd(nc, in_maps, core_ids=list(range(NCORES)),
                                        trace=trace)
    outv = np.empty((B, O), dtype=np.float32)
    for c in range(NCORES):
        outv[c * BC:(c + 1) * BC, :] = last_results.results[c]["out"].T
    return outv


# revision 32
# speedup vs baseline: 1.1795x; 1.1795x over previous
"""Trainium2 Bass kernel for NeuroVPR Vanilla SNN (3-layer LIF, T=3).

Data-parallel over batch: B=16384 -> 2048 per core x 8 cores.

Math (per timestep, per layer): v = (v_prev + h)/2; s = (v>=1); v *= (1-s).
Weights are scaled by 16 on host (keeps fp8-e4m3 values out of the subnormal
range), so PSUM holds 16h and we track W = 32*v:
    W_t = M_{t-1}/2 + psum_t    (psum = 16*h)
    s_t = (W_t >= 32)
    M_t = W_t * (W_t < 32) = (s_t < 0.5) * W_t
All scalings are powers of two, so spike decisions match the fp32 recurrence
up to matmul quantization error. Layer-2 membranes peak at 0.59 of threshold
under fp8 quantization (validated on host), so the output spikes are
insensitive to the fp8 rounding.

Matmuls run in fp8-e4m3 with perf_mode=DoubleRow: contraction tiles of 256
rows packed as [128 partitions, 2, free]; 2 MACs/PE/cycle. D is padded
2752->2816 = 11*256; pad row 2752 carries the L1 bias (x=1 there). The x
input is fp8, halving HBM traffic vs fp16.

LIF runs PSUM-resident everywhere: for t>=1, ScalarE preloads each bank
with M/2 and the matmul group accumulates on top with start=False (the
has_written bits survive from the previous full-bank group; L3 stationary
operands are padded to 128 rows so every group writes its full bank). The
bank then holds W directly: VectorE does s = (psum >= 32) and
m = (s < 0.5)*psum (one PSUM operand per instruction - the DVE cannot read
PSUM twice). At t=0 the membrane is zero, so no preload and start=True.

Schedule: the PE runs instructions in order, so layer-2/3 groups must never
bunch up behind PSUM-bank recycling (that head-blocks the queue, idles the
PE >3.4us, and HAM halves the PE clock). Each timestep's L2/L3 work is
split into 2-bank "pairs" interleaved as slots inside later L1 contraction
loops, as early as their spike inputs allow. GpSimd has no ALU or PSUM path
on this target; its DMA queue is kept empty (a queued DMA there causes a
12us DRAIN stall mid-kernel).
"""
import os
import numpy as np

B, T, D = 16384, 3, 2752
DP = 2816           # D padded to 11*256 (pad row 2752 = bias row)
H, O = 256, 100
NCORES = 8
BC = B // NCORES    # 2048
NB = 512            # psum block along batch
KT2 = DP // 256     # 11 double-row contraction tiles for L1
HB = BC // 2        # half-batch per L1 pass (1024)

_compiled = {}
last_results = None  # BassKernelResults of the most recent run (for profiling)


def _build(use_b2, use_b3):
    from contextlib import ExitStack
    import concourse.bass as bass
    import concourse.mybir as mybir
    import concourse.tile as tile
    from concourse import bacc

    f8, f16, f32 = mybir.dt.float8e4, mybir.dt.float16, mybir.dt.float32
    A = mybir.AluOpType
    DR = mybir.MatmulPerfMode.DoubleRow

    nc = bacc.Bacc("TRN2", target_bir_lowering=False, debug=False)
    x = nc.dram_tensor("x", [T, KT2, 128, 2, BC], f8, kind="ExternalInput").ap()
    # w1 free layout (i, kk, m): [p, i*(KT2*H) + kk*H + m] = 16*W1T[kk*256+i*128+p, m]
    w1 = nc.dram_tensor("w1", [128, 2 * KT2 * H], f8, kind="ExternalInput").ap()
    w2 = nc.dram_tensor("w2", [128, 2 * H], f8, kind="ExternalInput").ap()
    w3 = nc.dram_tensor("w3", [128, 2 * 128], f8, kind="ExternalInput").ap()
    if use_b2:
        b2 = nc.dram_tensor("b2", [1, H], f16, kind="ExternalInput").ap()
    if use_b3:
        b3 = nc.dram_tensor("b3", [1, O], f16, kind="ExternalInput").ap()
    out = nc.dram_tensor("out", [O, BC], f32, kind="ExternalOutput").ap()

    W1C = KT2 * H  # column span of one i-group in w1t

    with tile.TileContext(nc) as tc, ExitStack() as ctx:
        wp = ctx.enter_context(tc.tile_pool(name="wp", bufs=1))
        xp = ctx.enter_context(tc.tile_pool(name="xp", bufs=24))
        pp1 = ctx.enter_context(tc.tile_pool(name="pp1", bufs=5, space="PSUM"))
        pp23 = ctx.enter_context(tc.tile_pool(name="pp23", bufs=3, space="PSUM"))
        sp = ctx.enter_context(tc.tile_pool(name="sp", bufs=1))
        tp = ctx.enter_context(tc.tile_pool(name="tp", bufs=6))

        # Warm-up: a few dummy DoubleRow matmuls on zeroed tiles keep the PE
        # busy from the preamble until the first real matmul's x tile lands,
        # so the HAM clock gate opens early and every real matmul runs at
        # 2.4 GHz (cold-start otherwise costs ~5us at half clock). They run
        # at the cold clock (~630ns each); too many head-block the queue.
        zw = wp.tile([128, 256], f8)
        zx = wp.tile([128, 2 * NB], f8)
        nc.vector.memset(zw[:, :], 0.0)
        nc.vector.memset(zx[:, :], 0.0)
        zw3 = zw[:, :].rearrange("p (i m) -> p i m", i=2)
        zx3 = zx[:, :].rearrange("p (i n) -> p i n", i=2)
        zp = pp23.tile([128, NB], f32, tag="ps23", name="warmup")
        for _ in range(10):
            nc.tensor.matmul(zp[:, :], zw3, zx3, start=True, stop=True,
                             perf_mode=DR)

        # resident weights
        w1t = wp.tile([128, 2 * W1C], f8)
        # kk=0 chunks first on the sync queue so matmuls can start ASAP
        nc.sync.dma_start(out=w1t[:, 0:H], in_=w1[:, 0:H])
        nc.sync.dma_start(out=w1t[:, W1C:W1C + H], in_=w1[:, W1C:W1C + H])
        nc.scalar.dma_start(out=w1t[:, H:W1C], in_=w1[:, H:W1C])
        nc.scalar.dma_start(out=w1t[:, W1C + H:2 * W1C], in_=w1[:, W1C + H:2 * W1C])
        w2t = wp.tile([128, 2 * H], f8)
        nc.scalar.dma_start(out=w2t[:, :], in_=w2[:, :])
        w3t = wp.tile([128, 2 * 128], f8)
        nc.scalar.dma_start(out=w3t[:, :], in_=w3[:, :])
        if use_b2 or use_b3:
            ones = wp.tile([1, NB], f16)
            nc.vector.memset(ones[:, :], 1.0)
        if use_b2:
            b2t = wp.tile([1, H], f16)
            nc.scalar.dma_start(out=b2t[:, :], in_=b2[:, :])
        if use_b3:
            b3t = wp.tile([1, O], f16)
            nc.scalar.dma_start(out=b3t[:, :], in_=b3[:, :])

        w1v = w1t[:, :].rearrange("p (i c) -> p i c", i=2)
        w2v = w2t[:, :].rearrange("p (i m) -> p i m", i=2)
        w3v = w3t[:, :].rearrange("p (i m) -> p i m", i=2)

        # persistent state (M = 16*m, no init needed: t=0 skips the M read)
        m1 = [sp.tile([128, BC], f16, tag=f"m1_{h}", name=f"m1_{h}") for h in range(2)]
        m2 = [sp.tile([128, BC], f16, tag=f"m2_{h}", name=f"m2_{h}") for h in range(2)]
        m3 = sp.tile([128, BC], f16, tag="m3")
        # spikes, double-buffered by timestep parity; layout [p, i*BC + n]
        s1 = [sp.tile([128, 2 * BC], f8, tag=f"s1_{j}", name=f"s1_{j}") for j in range(2)]
        s2 = [sp.tile([128, 2 * BC], f8, tag=f"s2_{j}", name=f"s2_{j}") for j in range(2)]
        outsb = sp.tile([128, BC], f32, tag="outsb")

        s1v = [s1[j][:, :].rearrange("p (i n) -> p i n", i=2) for j in range(2)]
        s2v = [s2[j][:, :].rearrange("p (i n) -> p i n", i=2) for j in range(2)]

        def l2_pair(t, h, bp, pool, tag, bounce=False):
            """Two adjacent batch blocks of layer 2 via PSUM-preload.

            bounce=True (in-pass pends at T-1): ScalarE copies the banks into
            a fp16 tile so VectorE does one merged 1024-wide compare. Tail
            pairs keep the direct per-bank compare (shortest chain)."""
            def emit():
                ps = []
                for b in (bp, bp + 1):
                    p = pool.tile([128, NB], f32, tag=tag, name=f"ps2_{t}_{h}_{b}")
                    ps.append(p)
                    if t > 0:
                        nc.scalar.mul(p[:, :], m2[h][:, b * NB:(b + 1) * NB], 0.5)
                for j, b in enumerate((bp, bp + 1)):
                    first = t == 0
                    if use_b2:
                        nc.tensor.matmul(ps[j][:, :], b2t[0:1, h * 128:(h + 1) * 128],
                                         ones[0:1, :], start=first, stop=False,
                                         skip_group_check=not first)
                        first = False
                    nc.tensor.matmul(
                        ps[j][:, :], w2v[:, :, h * 128:(h + 1) * 128],
                        s1v[t % 2][:, :, b * NB:(b + 1) * NB],
                        start=first, stop=True, skip_group_check=t > 0,
                        perf_mode=DR)
                if bounce and t == T - 1:
                    wpr = tp.tile([128, 2 * NB], f16, tag="w", name="w")
                    for j in range(2):
                        nc.scalar.copy(wpr[:, j * NB:(j + 1) * NB], ps[j][:, :])
                    c0 = h * BC + bp * NB
                    nc.vector.tensor_scalar(s2[t % 2][:, c0:c0 + 2 * NB],
                                            wpr[:, :], 32.0, None, A.is_ge)
                    return
                for j, b in enumerate((bp, bp + 1)):
                    c0 = h * BC + b * NB
                    nc.vector.tensor_scalar(s2[t % 2][:, c0:c0 + NB], ps[j][:, :],
                                            32.0, None, A.is_ge)
                if t != T - 1:
                    for j, b in enumerate((bp, bp + 1)):
                        c0 = h * BC + b * NB
                        nc.vector.scalar_tensor_tensor(
                            m2[h][:, b * NB:(b + 1) * NB],
                            s2[t % 2][:, c0:c0 + NB], 0.5, ps[j][:, :],
                            A.is_lt, A.mult)
            return emit

        def l3_pair(t, bp, pool, tag):
            """Layer-3 pair; stationary padded to 128 rows so the matmul
            writes the full bank (keeps has_written set for later preloads).
            At t<T-1 only the membrane update is needed: ScalarE bounces the
            banks to a fp16 tile, VectorE does one merged 1024-wide update."""
            def emit():
                ps = []
                for b in (bp, bp + 1):
                    p = pool.tile([128, NB], f32, tag=tag, name=f"ps3_{t}_{b}")
                    ps.append(p)
                    if t > 0:
                        nc.scalar.mul(p[:O, :], m3[:O, b * NB:(b + 1) * NB], 0.5)
                for j, b in enumerate((bp, bp + 1)):
                    first = t == 0
                    if use_b3:
                        nc.tensor.matmul(ps[j][:O, :], b3t[0:1, :], ones[0:1, :],
                                         start=first, stop=False,
                                         skip_group_check=not first)
                        first = False
                    nc.tensor.matmul(
                        ps[j][:, :], w3v[:, :, :],
                        s2v[t % 2][:, :, b * NB:(b + 1) * NB],
                        start=first, stop=True, skip_group_check=t > 0,
                        perf_mode=DR)
                if t == T - 1:
                    for j, b in enumerate((bp, bp + 1)):
                        bs = slice(b * NB, (b + 1) * NB)
                        nc.vector.tensor_scalar(outsb[:O, bs], ps[j][:O, :],
                                                32.0, None, A.is_ge)
                        nc.sync.dma_start(out=out[:, bs], in_=outsb[:O, bs])
                else:
                    # m = (W<32)*W with one PSUM read per op: inverse-spike
                    # scratch ns = (psum < 32), then m = ns * psum
                    for j, b in enumerate((bp, bp + 1)):
                        bs = slice(b * NB, (b + 1) * NB)
                        ns = tp.tile([128, NB], f8, tag="ns", name="ns")
                        nc.vector.tensor_scalar(ns[:O, :], ps[j][:O, :],
                                                32.0, None, A.is_lt)
                        nc.vector.scalar_tensor_tensor(
                            m3[:O, bs], ns[:O, :], 0.0, ps[j][:O, :],
                            A.bypass, A.mult)
            return emit

        def l1_pass(t, half, pends=(), slots=()):
            """One half-batch L1 pass: 4 psum groups (2h x 2b), kk inner.
            Pending L2/L3 pair closures are emitted at the given kk slots.
            For t>=1 the banks are preloaded with M/2 and the whole
            accumulation group runs with start=False."""
            boff = half * HB
            pends = list(pends)
            slots = sorted(slots)[:len(pends)]
            ps1 = [[pp1.tile([128, NB], f32, tag="ps1", name=f"ps1_{t}_{half}_{h}_{b}")
                    for b in range(2)] for h in range(2)]
            last_t = t == T - 1
            if last_t:
                # no membrane write at T-1: preload M/2 so the bank holds W
                # after the matmuls and a single per-bank compare finishes
                # the pass (shortest possible tail chain)
                for h in range(2):
                    for b in range(2):
                        bs = slice(boff + b * NB, boff + (b + 1) * NB)
                        nc.scalar.mul(ps1[h][b][:, :], m1[h][:, bs], 0.5)
            for k in range(KT2):
                xt = xp.tile([128, 2 * HB], f8, tag="x", name="xt")
                xt3 = xt[:, :].rearrange("p (i n) -> p i n", i=2)
                nc.sync.dma_start(out=xt3, in_=x[t, k, :, :, boff:boff + HB])
                for h in range(2):
                    lhsT = w1v[:, :, k * H + h * 128: k * H + h * 128 + 128]
                    for b in range(2):
                        nc.tensor.matmul(
                            ps1[h][b][:, :], lhsT, xt3[:, :, b * NB:(b + 1) * NB],
                            start=(k == 0 and not last_t), stop=(k == KT2 - 1),
                            skip_group_check=last_t, perf_mode=DR)
                if slots and k == slots[0]:
                    slots.pop(0)
                    pends.pop(0)()
            if last_t:
                # per-bank spikes, b-major so the tail's l2 inputs finish first
                for b in range(2):
                    for h in range(2):
                        c0 = h * BC + boff + b * NB
                        nc.vector.tensor_scalar(s1[t % 2][:, c0:c0 + NB],
                                                ps1[h][b][:, :], 32.0, None,
                                                A.is_ge)
            else:
                # W into fp16 tiles first (releases all 4 banks quickly),
                # then merged 1024-wide spike/membrane ops
                wh = [tp.tile([128, 2 * NB], f16, tag="w", name="w")
                      for _ in range(2)]
                for h in range(2):
                    for b in range(2):
                        bs = slice(boff + b * NB, boff + (b + 1) * NB)
                        dst = wh[h][:, b * NB:(b + 1) * NB]
                        if t == 0:
                            nc.scalar.copy(dst, ps1[h][b][:, :])
                        else:
                            nc.vector.scalar_tensor_tensor(
                                dst, m1[h][:, bs], 0.5, ps1[h][b][:, :],
                                A.mult, A.add)
                for h in range(2):
                    c0 = h * BC + boff
                    nc.vector.tensor_scalar(s1[t % 2][:, c0:c0 + HB],
                                            wh[h][:, :], 32.0, None, A.is_ge)
                if t != T - 1:
                    for h in range(2):
                        wv = wh[h][:, :]
                        nc.vector.scalar_tensor_tensor(
                            m1[h][:, boff:boff + HB], wv, 32.0, wv,
                            A.is_lt, A.mult)
            for p in pends:  # leftovers (shouldn't happen with matched slots)
                p()

        # Pair placement: as early as spike inputs allow, so the final pass
        # carries little and the tail stays short.
        l1_pass(0, 0)
        l1_pass(0, 1)
        l1_pass(1, 0,
                [l2_pair(0, 0, 0, pp23, "ps23"),
                 l2_pair(0, 1, 0, pp23, "ps23"),
                 l2_pair(0, 0, 2, pp23, "ps23")], (2, 5, 8))
        l1_pass(1, 1,
                [l2_pair(0, 1, 2, pp23, "ps23"),
                 l3_pair(0, 0, pp23, "ps23"),
                 l3_pair(0, 2, pp23, "ps23")], (2, 5, 8))
        l1_pass(2, 0,
                [l2_pair(1, 0, 0, pp23, "ps23"),
                 l2_pair(1, 1, 0, pp23, "ps23"),
                 l2_pair(1, 0, 2, pp23, "ps23"),
                 l2_pair(1, 1, 2, pp23, "ps23")], (1, 3, 5, 8))
        l1_pass(2, 1,
                [l3_pair(1, 0, pp23, "ps23"),
                 l3_pair(1, 2, pp23, "ps23"),
                 l2_pair(2, 0, 0, pp23, "ps23"),
                 l2_pair(2, 1, 0, pp23, "ps23")], (1, 4, 6, 8))
        # tail
        l3_pair(2, 0, pp23, "ps23")()
        l2_pair(2, 0, 2, pp1, "ps1")()
        l2_pair(2, 1, 2, pp1, "ps1")()
        l3_pair(2, 2, pp23, "ps23")()

    nc.compile()
    return nc


def kernel(dvs, W1, b1, W2, b2, W3, b3):
    global last_results
    import ml_dtypes
    from concourse.bass_utils import run_bass_kernel_spmd

    f8 = ml_dtypes.float8_e4m3
    use_b2 = bool(np.any(b2))
    use_b3 = bool(np.any(b3))
    key = (use_b2, use_b3)
    if key not in _compiled:
        _compiled[key] = _build(use_b2, use_b3)
    nc = _compiled[key]

    one8 = np.float32(1.0).astype(f8).view(np.uint8).item()
    # x: [B, T, D] -> fp8 [T, KT2, 128, 2, B]; pad row D=2752 carries bias (x=1)
    Xq = np.empty((T, DP, B), np.uint8)
    for t in range(T):
        Xq[t, :D, :] = np.asarray(dvs[:, t, :]).astype(f8).view(np.uint8).T
        Xq[t, D, :] = one8
        Xq[t, D + 1:, :] = 0
    Xr = Xq.reshape(T, KT2, 2, 128, B).swapaxes(2, 3)  # [T, KT2, 128, 2, B] view

    # weights scaled by 16, packed [p, i, ...] for DoubleRow
    W1p = np.zeros((DP, H), np.float32)
    W1p[:D] = W1.T * 16.0
    W1p[D] = b1 * 16.0
    w1q = W1p.astype(f8).reshape(KT2, 2, 128, H)
    w1dr = np.ascontiguousarray(
        w1q.transpose(2, 1, 0, 3).reshape(128, 2 * KT2 * H))
    w2dr = np.ascontiguousarray(
        (W2.T * 16.0).astype(f8).reshape(2, 128, H).transpose(1, 0, 2).reshape(128, 2 * H))
    W3p = np.zeros((H, 128), np.float32)
    W3p[:, :O] = W3.T * 16.0
    w3dr = np.ascontiguousarray(
        W3p.astype(f8).reshape(2, 128, 128).transpose(1, 0, 2).reshape(128, 256))

    in_maps = []
    for c in range(NCORES):
        xc = np.ascontiguousarray(Xr[:, :, :, :, c * BC:(c + 1) * BC]).view(f8)
        m = {"x": xc, "w1": w1dr, "w2": w2dr, "w3": w3dr}
        if use_b2:
            m["b2"] = (b2 * 16.0).astype(np.float16).reshape(1, H)
        if use_b3:
            m["b3"] = (b3 * 16.0).astype(np.float16).reshape(1, O)
        in_maps.append(m)

    trace = bool(os.environ.get("SNN_TRACE"))
    last_results = run_bass_kernel_spmd(nc, in_maps, core_ids=list(range(NCORES)),
                                        trace=trace)
    outv = np.empty((B, O), dtype=np.float32)
    for c in range(NCORES):
        outv[c * BC:(c + 1) * BC, :] = last_results.results[c]["out"].T
    return outv
